# revision 2
# baseline (speedup 1.0000x reference)
"""BiGCN (bidirectional 2-layer GCN over many small graphs) on 8 Trainium2 cores.

v2: PE runs only the essential matmuls; everything else is folded away.
  - Host precomputes normalized adjacency At = (D^-1/2 (A+I) D^-1/2)^T (fp16)
    and rvec = X[roots] @ [W2r_td | W2r_bu]  (per-graph root feature row).
  - Y = X @ [W1_td|W1_bu]          6 MM  N=512   (node-major)
  - h^T = relu(Y_chunk^T @ At)     4 MM  N=128   (feature-major; no transposes)
  - Z = h @ W2h (+ rvec via DVE)   4 MM  N=256   (node-major)
  - H2^T chunks = Z_chunk^T @ At   4 MM  N=128   (feature-major)
  - mean readout: fused into the relu cast via accum_out (free-dim node sums)
  - root readout: column 0 of h^T, stashed per graph, transposed once at end
  - rank-1 root term: rvec broadcast across partitions on GpSimd, added in the
    zn cast on DVE. No PE work.
All matmuls fp16.
"""

import numpy as np

import concourse.bass as bass
import concourse.tile as tile
from concourse import bacc, mybir
from concourse.bass_utils import run_bass_kernel_spmd
from concourse.masks import make_identity

N_GRAPHS = 256
N_PER_G = 128
IN_FEATS = 768
H_FEATS = 256
N_CORES = 8
G_PER_CORE = N_GRAPHS // N_CORES            # 32
NODES_PER_CORE = G_PER_CORE * N_PER_G       # 4096
KCH = IN_FEATS // 128                       # 6 feature chunks

MM_DT = mybir.dt.float16
F32 = mybir.dt.float32
AF = mybir.ActivationFunctionType
OP = mybir.AluOpType


def build_adj(src, dst, n, G):
    """Normalized adjacency, transposed: At[g, j, i] = norm_j*norm_i*A[i, j]
    where A[i, j] = #edges j->i (self-loops are in the edge list)."""
    src = np.asarray(src, np.int64)
    dst = np.asarray(dst, np.int64)
    N = n * G
    if not np.array_equal(src // n, dst // n):
        raise ValueError("cross-graph edge found; contiguous-block sharding invalid")
    deg = np.bincount(dst, minlength=N).astype(np.float64)
    norm = 1.0 / np.sqrt(np.maximum(deg, 1.0))
    w = norm[src] * norm[dst]
    g = dst // n
    idx = g * (n * n) + (src - g * n) * n + (dst - g * n)
    at = np.bincount(idx, weights=w, minlength=G * n * n)
    return at.reshape(G, n, n).astype(np.float32)


# ----------------------------------------------------------------------------
# Device program (SPMD; one core's shard)
# ----------------------------------------------------------------------------

def build_program(has_bias):
    nc = bacc.Bacc("TRN2", target_bir_lowering=False, debug=False,
                   num_devices=N_CORES)

    def din(name, shape, dt=MM_DT):
        return nc.dram_tensor(name, shape, dt, kind="ExternalInput").ap()

    xt = din("xt", [128, G_PER_CORE, KCH, 128])          # X^T chunked per graph
    at_d = din("at_d", [128, G_PER_CORE, 2, 128])        # normalized adj^T
    w1p = din("w1p", [128, KCH, 2 * H_FEATS])            # [W1_td | W1_bu]
    w2h = din("w2h", [128, 2, 2, H_FEATS])               # W2h per (branch, chunk)
    rvrow = din("rvrow", [1, G_PER_CORE * 2 * H_FEATS], F32)  # host rvec rows
    if has_bias:
        b1c = din("b1c", [128, 4], F32)                  # b1 per (branch,chunk)
        b2c = din("b2c", [128, 4], F32)                  # b2 per (branch,chunk)
    out = nc.dram_tensor("out", [G_PER_CORE, 4 * H_FEATS], F32,
                         kind="ExternalOutput").ap()

    with tile.TileContext(nc) as tc:
        with (
            tc.tile_pool(name="const", bufs=1) as const,
            tc.tile_pool(name="xin", bufs=3) as xin,
            tc.tile_pool(name="atp", bufs=3) as atp,
            tc.tile_pool(name="ybp", bufs=2) as ybp,
            tc.tile_pool(name="htp", bufs=2) as htp,
            tc.tile_pool(name="znp", bufs=2) as znp,
            tc.tile_pool(name="rvf", bufs=2) as rvf,
            tc.tile_pool(name="scr", bufs=2) as scr,
            tc.tile_pool(name="psA", bufs=2, space="PSUM") as psA,
            tc.tile_pool(name="psH", bufs=2, space="PSUM") as psH,
            tc.tile_pool(name="psZ", bufs=2, space="PSUM") as psZ,
            tc.tile_pool(name="psT", bufs=2, space="PSUM") as psT,
        ):
            # ---- constants -------------------------------------------------
            identity_f32 = const.tile([128, 128], F32)
            make_identity(nc, identity_f32[:])

            w1p_sb = const.tile([128, KCH, 2 * H_FEATS], MM_DT)
            nc.sync.dma_start(w1p_sb[:], w1p)
            w2h_sb = const.tile([128, 2, 2, H_FEATS], MM_DT)
            nc.sync.dma_start(w2h_sb[:], w2h)
            rvrow_sb = const.tile([1, G_PER_CORE * 2 * H_FEATS], F32)
            nc.sync.dma_start(rvrow_sb[:], rvrow)
            if has_bias:
                b1c_sb = const.tile([128, 4], F32)
                nc.sync.dma_start(b1c_sb[:], b1c)
                b2c_sb = const.tile([128, 4], F32)
                nc.sync.dma_start(b2c_sb[:], b2c)

            # per-graph readout stashes (feature-major), transposed at the end
            roots_sb = const.tile([128, 4, G_PER_CORE], F32)
            means_sb = const.tile([128, 4, G_PER_CORE], F32)

            # ---- main loop over this core's graphs -------------------------
            for g in range(G_PER_CORE):
                xt_tile = xin.tile([128, KCH, 128], MM_DT, tag="xt")
                nc.sync.dma_start(xt_tile[:], xt[:, g])
                at_sb = atp.tile([128, 2, 128], MM_DT, tag="at")
                nc.sync.dma_start(at_sb[:], at_d[:, g])
                # rvec row -> all 128 partitions (GpSimd; SBUF->SBUF)
                rvfull = rvf.tile([128, 2 * H_FEATS], F32, tag="rvfull")
                nc.gpsimd.partition_broadcast(
                    rvfull[:], rvrow_sb[0:1, g * 2 * H_FEATS:(g + 1) * 2 * H_FEATS])

                # Y = X @ [W1_td | W1_bu]   (node-major [128, 512])
                ps_y = psA.tile([128, 2 * H_FEATS], F32, tag="psA")
                for k in range(KCH):
                    nc.tensor.matmul(ps_y[:], xt_tile[:, k, :], w1p_sb[:, k, :],
                                     start=(k == 0), stop=(k == KCH - 1))
                yb = ybp.tile([128, 2 * H_FEATS], MM_DT, tag="yb")
                nc.vector.tensor_copy(yb[:], ps_y[:])

                # h^T chunks = (Y chunk)^T @ At_b   (feature-major)
                ps_h = psH.tile([128, 4, 128], F32, tag="psH")
                for b in (0, 1):
                    for c in (0, 1):
                        fo = b * 2 + c
                        nc.tensor.matmul(ps_h[:, fo, :],
                                         yb[:, fo * 128:(fo + 1) * 128],
                                         at_sb[:, b, :])
                ht = htp.tile([128, 4, 128], MM_DT, tag="ht")
                if has_bias:
                    for fo in range(4):
                        nc.scalar.activation(ht[:, fo, :], ps_h[:, fo, :],
                                             AF.Relu, bias=b1c_sb[:, fo:fo + 1])
                else:
                    nc.scalar.activation(ht[:], ps_h[:], AF.Relu)
                # stash roots' h (feature-major: column 0 of each chunk)
                nc.gpsimd.tensor_copy(roots_sb[:, :, g:g + 1], ht[:, :, 0:1])

                # Z = h @ W2h   (node-major [128, 512]); rvec added in the cast
                ps_z = psZ.tile([128, 2 * H_FEATS], F32, tag="psZ")
                for b in (0, 1):
                    col = slice(b * H_FEATS, (b + 1) * H_FEATS)
                    for c in (0, 1):
                        nc.tensor.matmul(ps_z[:, col], ht[:, b * 2 + c, :],
                                         w2h_sb[:, b, c, :],
                                         start=(c == 0), stop=(c == 1))
                zn = znp.tile([128, 2 * H_FEATS], MM_DT, tag="zn")
                nc.vector.tensor_tensor(zn[:], ps_z[:], rvfull[:], OP.add)

                # H2^T chunks = (Z chunk)^T @ At_b  (feature-major); the relu
                # cast accumulates node sums per feature => mean readout free
                ps_t = psT.tile([128, 4, 128], F32, tag="psT")
                for b in (0, 1):
                    for c in (0, 1):
                        fo = b * 2 + c
                        nc.tensor.matmul(ps_t[:, fo, :],
                                         zn[:, fo * 128:(fo + 1) * 128],
                                         at_sb[:, b, :])
                h2s = scr.tile([128, 4, 128], MM_DT, tag="h2s")
                for fo in range(4):
                    if has_bias:
                        # bias+relu+sum needs two ALU stages before the
                        # reduce, which DVE can't express -> all on ACT
                        nc.scalar.activation(
                            h2s[:, fo, :], ps_t[:, fo, :], AF.Relu,
                            bias=b2c_sb[:, fo:fo + 1],
                            accum_out=means_sb[:, fo, g:g + 1])
                    elif fo < 2:
                        # DVE reduce form: out = (in0 op0 s1);
                        # accum_out = reduce(out, op1, init=s2)
                        nc.vector.tensor_scalar(
                            h2s[:, fo, :], ps_t[:, fo, :], 0.0, 0.0,
                            OP.max, OP.add,
                            accum_out=means_sb[:, fo, g:g + 1])
                    else:
                        nc.scalar.activation(
                            h2s[:, fo, :], ps_t[:, fo, :], AF.Relu,
                            accum_out=means_sb[:, fo, g:g + 1])

            # ---- readouts: transpose [128, 32] -> [32, 128] on PE ----------
            # means carry node SUMS; scale by 1/128 in the PSUM->SBUF copy.
            rootsT = const.tile([G_PER_CORE, 4, 128], F32)
            meansT = const.tile([G_PER_CORE, 4, 128], F32)
            ps_r = psH.tile([128, 4, 128], F32, tag="psH")
            ps_m = psT.tile([128, 4, 128], F32, tag="psT")
            for fo in range(4):
                nc.tensor.transpose(ps_r[0:G_PER_CORE, fo, :],
                                    roots_sb[:, fo, :], identity_f32[:])
                nc.tensor.transpose(ps_m[0:G_PER_CORE, fo, :],
                                    means_sb[:, fo, :], identity_f32[:])
            nc.scalar.copy(rootsT[:], ps_r[0:G_PER_CORE])
            # sums are >= 0, so Relu(x * 1/128) is an exact mean
            nc.vector.tensor_scalar(meansT[:], ps_m[0:G_PER_CORE],
                                    1.0 / N_PER_G, None, OP.mult)
            nc.sync.dma_start(out[:, 0:H_FEATS], meansT[:, 0:2, :])
            nc.sync.dma_start(out[:, 2 * H_FEATS:3 * H_FEATS], meansT[:, 2:4, :])
            nc.sync.dma_start(out[:, H_FEATS:2 * H_FEATS], rootsT[:, 0:2, :])
            nc.sync.dma_start(out[:, 3 * H_FEATS:4 * H_FEATS], rootsT[:, 2:4, :])

    nc.compile()
    return nc


# ----------------------------------------------------------------------------
# Host entry point
# ----------------------------------------------------------------------------

def _prep(inputs, w1_td, b1_td, w2_td, b2_td, w1_bu, b1_bu, w2_bu, b2_bu,
          td_src, td_dst, bu_src, bu_dst, nodes_per_graph):
    n = int(nodes_per_graph)
    X = np.asarray(inputs, np.float32)
    N = X.shape[0]
    G = N // n
    assert (n, G, X.shape[1]) == (N_PER_G, N_GRAPHS, IN_FEATS), \
        f"unexpected shapes {X.shape} n={n}"

    at_td = build_adj(td_src, td_dst, n, G)   # [G, 128, 128] f32
    at_bu = build_adj(bu_src, bu_dst, n, G)

    w1p = np.concatenate([np.asarray(w1_td, np.float32),
                          np.asarray(w1_bu, np.float32)], axis=1)
    w1p_l = np.ascontiguousarray(
        w1p.reshape(KCH, 128, 2 * H_FEATS).transpose(1, 0, 2)).astype(np.float16)
    w2_td = np.asarray(w2_td, np.float32)
    w2_bu = np.asarray(w2_bu, np.float32)
    w2h = np.stack([w2_td[:H_FEATS].reshape(2, 128, H_FEATS),
                    w2_bu[:H_FEATS].reshape(2, 128, H_FEATS)], axis=0)
    w2h_l = np.ascontiguousarray(w2h.transpose(2, 0, 1, 3)).astype(np.float16)
    w2rp = np.concatenate([w2_td[H_FEATS:], w2_bu[H_FEATS:]], axis=1)  # [768,512]
    rv_all = (X[::n] @ w2rp).astype(np.float32)                        # [G, 512]

    biases = [np.asarray(b, np.float32) for b in (b1_td, b2_td, b1_bu, b2_bu)]
    has_bias = any(np.any(b != 0) for b in biases)

    in_maps = []
    for c in range(N_CORES):
        gs = slice(c * G_PER_CORE, (c + 1) * G_PER_CORE)
        ns = slice(c * NODES_PER_CORE, (c + 1) * NODES_PER_CORE)
        Xc = X[ns]
        xt_l = np.ascontiguousarray(
            Xc.reshape(G_PER_CORE, 128, KCH, 128).transpose(3, 0, 2, 1)
        ).astype(np.float16)
        at_c = np.stack([at_td[gs], at_bu[gs]], axis=0)  # [b, g, j, i]
        at_l = np.ascontiguousarray(at_c.transpose(2, 1, 0, 3)).astype(np.float16)
        m = {
            "xt": xt_l,
            "at_d": at_l,
            "w1p": w1p_l,
            "w2h": w2h_l,
            "rvrow": np.ascontiguousarray(rv_all[gs].reshape(1, -1)),
        }
        if has_bias:
            b1cat = np.concatenate([biases[0], biases[2]])       # [512]
            m["b1c"] = np.ascontiguousarray(b1cat.reshape(4, 128).T)
            b2cat = np.concatenate([biases[1], biases[3]])       # [512]
            m["b2c"] = np.ascontiguousarray(b2cat.reshape(4, 128).T)
        in_maps.append(m)
    return in_maps, has_bias


_PROGRAM_CACHE = {}


def _get_program(key):
    if key not in _PROGRAM_CACHE:
        _PROGRAM_CACHE[key] = build_program(key)
    return _PROGRAM_CACHE[key]


def kernel(trace=False, tmpdir=None, _return_raw=False, **inputs):
    in_maps, has_bias = _prep(**inputs)
    nc = _get_program(has_bias)
    res = run_bass_kernel_spmd(nc, in_maps, list(range(N_CORES)),
                               trace=trace, tmpdir=tmpdir)
    out = np.concatenate([res.results[i]["out"] for i in range(N_CORES)], axis=0)
    if _return_raw:
        return out, res
    return out


# revision 3
# speedup vs baseline: 1.1691x; 1.1691x over previous
"""BiGCN (bidirectional 2-layer GCN over many small graphs) on 8 Trainium2 cores.

v2: PE runs only the essential matmuls; everything else is folded away.
  - Host precomputes normalized adjacency At = (D^-1/2 (A+I) D^-1/2)^T (fp16)
    and rvec = X[roots] @ [W2r_td | W2r_bu]  (per-graph root feature row).
  - Y = X @ [W1_td|W1_bu]          6 MM  N=512   (node-major)
  - h^T = relu(Y_chunk^T @ At)     4 MM  N=128   (feature-major; no transposes)
  - Z = h @ W2h (+ rvec via DVE)   4 MM  N=256   (node-major)
  - H2^T chunks = Z_chunk^T @ At   4 MM  N=128   (feature-major)
  - mean readout: fused into the relu cast via accum_out (free-dim node sums)
  - root readout: column 0 of h^T, stashed per graph, transposed once at end
  - rank-1 root term: rvec broadcast across partitions on GpSimd, added in the
    zn cast on DVE. No PE work.
All matmuls fp16.
"""

import numpy as np

import concourse.bass as bass
import concourse.tile as tile
from concourse import bacc, mybir
from concourse.bass_utils import run_bass_kernel_spmd
from concourse.masks import make_identity

N_GRAPHS = 256
N_PER_G = 128
IN_FEATS = 768
H_FEATS = 256
N_CORES = 8
G_PER_CORE = N_GRAPHS // N_CORES            # 32
NODES_PER_CORE = G_PER_CORE * N_PER_G       # 4096
KCH = IN_FEATS // 128                       # 6 feature chunks

MM_DT = mybir.dt.float16
F32 = mybir.dt.float32
AF = mybir.ActivationFunctionType
OP = mybir.AluOpType


def build_adj(src, dst, n, G):
    """Normalized adjacency, transposed: At[g, j, i] = norm_j*norm_i*A[i, j]
    where A[i, j] = #edges j->i (self-loops are in the edge list)."""
    src = np.asarray(src, np.int64)
    dst = np.asarray(dst, np.int64)
    N = n * G
    if not np.array_equal(src // n, dst // n):
        raise ValueError("cross-graph edge found; contiguous-block sharding invalid")
    deg = np.bincount(dst, minlength=N).astype(np.float64)
    norm = 1.0 / np.sqrt(np.maximum(deg, 1.0))
    w = norm[src] * norm[dst]
    g = dst // n
    idx = g * (n * n) + (src - g * n) * n + (dst - g * n)
    at = np.bincount(idx, weights=w, minlength=G * n * n)
    return at.reshape(G, n, n).astype(np.float32)


# ----------------------------------------------------------------------------
# Device program (SPMD; one core's shard)
# ----------------------------------------------------------------------------

def build_program(has_bias):
    nc = bacc.Bacc("TRN2", target_bir_lowering=False, debug=False,
                   num_devices=N_CORES)

    def din(name, shape, dt=MM_DT):
        return nc.dram_tensor(name, shape, dt, kind="ExternalInput").ap()

    xt = din("xt", [128, G_PER_CORE, KCH, 128])          # X^T chunked per graph
    at_d = din("at_d", [128, G_PER_CORE, 2, 128])        # normalized adj^T
    w1p = din("w1p", [128, KCH, 2 * H_FEATS])            # [W1_td | W1_bu]
    w2h = din("w2h", [128, 2, 2, H_FEATS])               # W2h per (branch, chunk)
    rvrow = din("rvrow", [1, G_PER_CORE * 2 * H_FEATS], F32)  # host rvec rows
    if has_bias:
        b1c = din("b1c", [128, 4], F32)                  # b1 per (branch,chunk)
        b2c = din("b2c", [128, 4], F32)                  # b2 per (branch,chunk)
    out = nc.dram_tensor("out", [G_PER_CORE, 4 * H_FEATS], F32,
                         kind="ExternalOutput").ap()

    with tile.TileContext(nc) as tc:
        with (
            tc.tile_pool(name="const", bufs=1) as const,
            tc.tile_pool(name="xin", bufs=3) as xin,
            tc.tile_pool(name="atp", bufs=3) as atp,
            tc.tile_pool(name="ybp", bufs=2) as ybp,
            tc.tile_pool(name="htp", bufs=2) as htp,
            tc.tile_pool(name="znp", bufs=2) as znp,
            tc.tile_pool(name="rvf", bufs=2) as rvf,
            tc.tile_pool(name="scr", bufs=2) as scr,
            tc.tile_pool(name="psA", bufs=2, space="PSUM") as psA,
            tc.tile_pool(name="psH", bufs=2, space="PSUM") as psH,
            tc.tile_pool(name="psZ", bufs=2, space="PSUM") as psZ,
            tc.tile_pool(name="psT", bufs=2, space="PSUM") as psT,
        ):
            # ---- constants -------------------------------------------------
            identity_f32 = const.tile([128, 128], F32)
            make_identity(nc, identity_f32[:])
            # warm the PE p-state while startup DMAs are in flight
            ps_warm = psT.tile([128, 4, 128], F32, tag="psT")
            for i in range(14):
                nc.tensor.transpose(ps_warm[:, i % 4, :], identity_f32[:],
                                    identity_f32[:])

            w1p_sb = const.tile([128, KCH, 2 * H_FEATS], MM_DT)
            nc.sync.dma_start(w1p_sb[:], w1p)
            w2h_sb = const.tile([128, 2, 2, H_FEATS], MM_DT)
            nc.sync.dma_start(w2h_sb[:], w2h)
            rvrow_sb = const.tile([1, G_PER_CORE * 2 * H_FEATS], F32)
            nc.sync.dma_start(rvrow_sb[:], rvrow)
            if has_bias:
                b1c_sb = const.tile([128, 4], F32)
                nc.sync.dma_start(b1c_sb[:], b1c)
                b2c_sb = const.tile([128, 4], F32)
                nc.sync.dma_start(b2c_sb[:], b2c)

            # per-graph readout stashes (feature-major), transposed and
            # flushed to DRAM in two halves (mid-loop and at the end)
            G = G_PER_CORE
            roots_sb = const.tile([128, 4, G_PER_CORE], F32)
            means_sb = const.tile([128, 4, G_PER_CORE], F32)
            flushT = {}

            def flush_readout(lo, hi):
                n = hi - lo
                ps_r = psH.tile([128, 4, 128], F32, tag="psH")
                ps_m = psT.tile([128, 4, 128], F32, tag="psT")
                for fo in range(4):
                    nc.tensor.transpose(ps_r[0:n, fo, :],
                                        roots_sb[:, fo, lo:hi], identity_f32[:])
                    nc.tensor.transpose(ps_m[0:n, fo, :],
                                        means_sb[:, fo, lo:hi], identity_f32[:])
                rootsT = const.tile([n, 4, 128], F32, name=f"rootsT{lo}")
                meansT = const.tile([n, 4, 128], F32, name=f"meansT{lo}")
                flushT[lo] = (rootsT, meansT)
                nc.scalar.copy(rootsT[:], ps_r[0:n])
                # sums are >= 0, so the plain scale gives the exact mean
                nc.vector.tensor_scalar(meansT[:], ps_m[0:n],
                                        1.0 / N_PER_G, None, OP.mult)
                nc.sync.dma_start(out[lo:hi, 0:H_FEATS], meansT[:, 0:2, :])
                nc.sync.dma_start(out[lo:hi, 2 * H_FEATS:3 * H_FEATS],
                                  meansT[:, 2:4, :])
                nc.sync.dma_start(out[lo:hi, H_FEATS:2 * H_FEATS],
                                  rootsT[:, 0:2, :])
                nc.sync.dma_start(out[lo:hi, 3 * H_FEATS:4 * H_FEATS],
                                  rootsT[:, 2:4, :])

            # ---- main loop over this core's graphs -------------------------
            for g in range(G_PER_CORE):
                xt_tile = xin.tile([128, KCH, 128], MM_DT, tag="xt")
                nc.sync.dma_start(xt_tile[:], xt[:, g])
                at_sb = atp.tile([128, 2, 128], MM_DT, tag="at")
                nc.sync.dma_start(at_sb[:], at_d[:, g])
                # rvec row -> all 128 partitions (GpSimd; SBUF->SBUF)
                rvfull = rvf.tile([128, 2 * H_FEATS], F32, tag="rvfull")
                nc.gpsimd.partition_broadcast(
                    rvfull[:], rvrow_sb[0:1, g * 2 * H_FEATS:(g + 1) * 2 * H_FEATS])

                # Y = X @ [W1_td | W1_bu]   (node-major [128, 512])
                ps_y = psA.tile([128, 2 * H_FEATS], F32, tag="psA")
                for k in range(KCH):
                    nc.tensor.matmul(ps_y[:], xt_tile[:, k, :], w1p_sb[:, k, :],
                                     start=(k == 0), stop=(k == KCH - 1))
                yb = ybp.tile([128, 2 * H_FEATS], MM_DT, tag="yb")
                nc.vector.tensor_copy(yb[:], ps_y[:])

                # h^T chunks = (Y chunk)^T @ At_b   (feature-major)
                ps_h = psH.tile([128, 4, 128], F32, tag="psH")
                for b in (0, 1):
                    for c in (0, 1):
                        fo = b * 2 + c
                        nc.tensor.matmul(ps_h[:, fo, :],
                                         yb[:, fo * 128:(fo + 1) * 128],
                                         at_sb[:, b, :])
                ht = htp.tile([128, 4, 128], MM_DT, tag="ht")
                if has_bias:
                    for fo in range(4):
                        nc.scalar.activation(ht[:, fo, :], ps_h[:, fo, :],
                                             AF.Relu, bias=b1c_sb[:, fo:fo + 1])
                else:
                    nc.scalar.activation(ht[:], ps_h[:], AF.Relu)
                # stash roots' h (feature-major: column 0 of each chunk)
                nc.gpsimd.tensor_copy(roots_sb[:, :, g:g + 1], ht[:, :, 0:1])

                # Z = h @ W2h   (node-major [128, 512]); rvec added in the cast
                ps_z = psZ.tile([128, 2 * H_FEATS], F32, tag="psZ")
                for b in (0, 1):
                    col = slice(b * H_FEATS, (b + 1) * H_FEATS)
                    for c in (0, 1):
                        nc.tensor.matmul(ps_z[:, col], ht[:, b * 2 + c, :],
                                         w2h_sb[:, b, c, :],
                                         start=(c == 0), stop=(c == 1))
                zn = znp.tile([128, 2 * H_FEATS], MM_DT, tag="zn")
                nc.vector.tensor_tensor(zn[:], ps_z[:], rvfull[:], OP.add)

                # H2^T chunks = (Z chunk)^T @ At_b  (feature-major); the relu
                # cast accumulates node sums per feature => mean readout free
                ps_t = psT.tile([128, 4, 128], F32, tag="psT")
                for b in (0, 1):
                    for c in (0, 1):
                        fo = b * 2 + c
                        nc.tensor.matmul(ps_t[:, fo, :],
                                         zn[:, fo * 128:(fo + 1) * 128],
                                         at_sb[:, b, :])
                h2s = scr.tile([128, 4, 128], MM_DT, tag="h2s")
                for fo in range(4):
                    if has_bias:
                        # bias+relu+sum needs two ALU stages before the
                        # reduce, which DVE can't express -> all on ACT
                        nc.scalar.activation(
                            h2s[:, fo, :], ps_t[:, fo, :], AF.Relu,
                            bias=b2c_sb[:, fo:fo + 1],
                            accum_out=means_sb[:, fo, g:g + 1])
                    elif fo < 2:
                        # DVE reduce form: out = (in0 op0 s1);
                        # accum_out = reduce(out, op1, init=s2)
                        nc.vector.tensor_scalar(
                            h2s[:, fo, :], ps_t[:, fo, :], 0.0, 0.0,
                            OP.max, OP.add,
                            accum_out=means_sb[:, fo, g:g + 1])
                    else:
                        nc.scalar.activation(
                            h2s[:, fo, :], ps_t[:, fo, :], AF.Relu,
                            accum_out=means_sb[:, fo, g:g + 1])
                if g == G // 2 - 1:
                    flush_readout(0, G // 2)
                elif g == G - 1:
                    flush_readout(G // 2, G)



    nc.compile()
    return nc


# ----------------------------------------------------------------------------
# Host entry point
# ----------------------------------------------------------------------------

def _prep(inputs, w1_td, b1_td, w2_td, b2_td, w1_bu, b1_bu, w2_bu, b2_bu,
          td_src, td_dst, bu_src, bu_dst, nodes_per_graph):
    n = int(nodes_per_graph)
    X = np.asarray(inputs, np.float32)
    N = X.shape[0]
    G = N // n
    assert (n, G, X.shape[1]) == (N_PER_G, N_GRAPHS, IN_FEATS), \
        f"unexpected shapes {X.shape} n={n}"

    at_td = build_adj(td_src, td_dst, n, G)   # [G, 128, 128] f32
    at_bu = build_adj(bu_src, bu_dst, n, G)

    w1p = np.concatenate([np.asarray(w1_td, np.float32),
                          np.asarray(w1_bu, np.float32)], axis=1)
    w1p_l = np.ascontiguousarray(
        w1p.reshape(KCH, 128, 2 * H_FEATS).transpose(1, 0, 2)).astype(np.float16)
    w2_td = np.asarray(w2_td, np.float32)
    w2_bu = np.asarray(w2_bu, np.float32)
    w2h = np.stack([w2_td[:H_FEATS].reshape(2, 128, H_FEATS),
                    w2_bu[:H_FEATS].reshape(2, 128, H_FEATS)], axis=0)
    w2h_l = np.ascontiguousarray(w2h.transpose(2, 0, 1, 3)).astype(np.float16)
    w2rp = np.concatenate([w2_td[H_FEATS:], w2_bu[H_FEATS:]], axis=1)  # [768,512]
    rv_all = (X[::n] @ w2rp).astype(np.float32)                        # [G, 512]

    biases = [np.asarray(b, np.float32) for b in (b1_td, b2_td, b1_bu, b2_bu)]
    has_bias = any(np.any(b != 0) for b in biases)

    in_maps = []
    for c in range(N_CORES):
        gs = slice(c * G_PER_CORE, (c + 1) * G_PER_CORE)
        ns = slice(c * NODES_PER_CORE, (c + 1) * NODES_PER_CORE)
        Xc = X[ns]
        xt_l = np.ascontiguousarray(
            Xc.reshape(G_PER_CORE, 128, KCH, 128).transpose(3, 0, 2, 1)
        ).astype(np.float16)
        at_c = np.stack([at_td[gs], at_bu[gs]], axis=0)  # [b, g, j, i]
        at_l = np.ascontiguousarray(at_c.transpose(2, 1, 0, 3)).astype(np.float16)
        m = {
            "xt": xt_l,
            "at_d": at_l,
            "w1p": w1p_l,
            "w2h": w2h_l,
            "rvrow": np.ascontiguousarray(rv_all[gs].reshape(1, -1)),
        }
        if has_bias:
            b1cat = np.concatenate([biases[0], biases[2]])       # [512]
            m["b1c"] = np.ascontiguousarray(b1cat.reshape(4, 128).T)
            b2cat = np.concatenate([biases[1], biases[3]])       # [512]
            m["b2c"] = np.ascontiguousarray(b2cat.reshape(4, 128).T)
        in_maps.append(m)
    return in_maps, has_bias


_PROGRAM_CACHE = {}


def _get_program(key):
    if key not in _PROGRAM_CACHE:
        _PROGRAM_CACHE[key] = build_program(key)
    return _PROGRAM_CACHE[key]


def kernel(trace=False, tmpdir=None, _return_raw=False, **inputs):
    in_maps, has_bias = _prep(**inputs)
    nc = _get_program(has_bias)
    res = run_bass_kernel_spmd(nc, in_maps, list(range(N_CORES)),
                               trace=trace, tmpdir=tmpdir)
    out = np.concatenate([res.results[i]["out"] for i in range(N_CORES)], axis=0)
    if _return_raw:
        return out, res
    return out


# revision 4
# speedup vs baseline: 1.2136x; 1.0381x over previous
"""BiGCN (bidirectional 2-layer GCN over many small graphs) on 8 Trainium2 cores.

PE runs only the essential matmuls; everything else is folded away.
  - Host precomputes normalized adjacency At = (D^-1/2 (A+I) D^-1/2)^T (fp16)
    and rvec = X[roots] @ [W2r_td | W2r_bu]  (per-graph root feature row).
  - Y = X @ [W1_td|W1_bu]          6 MM  N=512   (node-major)
  - h^T = relu(Y_chunk^T @ At)     4 MM  N=128   (feature-major; no transposes)
  - Z = h @ W2h (+ rvec via DVE)   4 MM  N=256   (node-major)
  - H2^T chunks = Z_chunk^T @ At   4 MM  N=128   (feature-major)
  - mean readout: fused into the relu cast via accum_out (free-dim node sums)
  - root readout: column 0 of h^T, stashed per graph, transposed once at end
  - rank-1 root term: rvec broadcast across partitions on GpSimd, added in the
    zn cast on DVE. No PE work.
All matmuls fp16.
"""

import numpy as np

import concourse.bass as bass
import concourse.tile as tile
from concourse import bacc, mybir
from concourse.bass_utils import run_bass_kernel_spmd
from concourse.masks import make_identity

N_GRAPHS = 256
N_PER_G = 128
IN_FEATS = 768
H_FEATS = 256
N_CORES = 8
G_PER_CORE = N_GRAPHS // N_CORES            # 32
NODES_PER_CORE = G_PER_CORE * N_PER_G       # 4096
KCH = IN_FEATS // 128                       # 6 feature chunks

MM_DT = mybir.dt.float16
F32 = mybir.dt.float32
AF = mybir.ActivationFunctionType
OP = mybir.AluOpType


def build_adj(src, dst, n, G):
    """Normalized adjacency, transposed: At[g, j, i] = norm_j*norm_i*A[i, j]
    where A[i, j] = #edges j->i (self-loops are in the edge list)."""
    src = np.asarray(src, np.int64)
    dst = np.asarray(dst, np.int64)
    N = n * G
    if not np.array_equal(src // n, dst // n):
        raise ValueError("cross-graph edge found; contiguous-block sharding invalid")
    deg = np.bincount(dst, minlength=N).astype(np.float64)
    norm = 1.0 / np.sqrt(np.maximum(deg, 1.0))
    w = norm[src] * norm[dst]
    g = dst // n
    idx = g * (n * n) + (src - g * n) * n + (dst - g * n)
    at = np.bincount(idx, weights=w, minlength=G * n * n)
    return at.reshape(G, n, n).astype(np.float32)


# ----------------------------------------------------------------------------
# Device program (SPMD; one core's shard)
# ----------------------------------------------------------------------------

def build_program(has_bias):
    nc = bacc.Bacc("TRN2", target_bir_lowering=False, debug=False,
                   num_devices=N_CORES)

    def din(name, shape, dt=MM_DT):
        return nc.dram_tensor(name, shape, dt, kind="ExternalInput").ap()

    xt = din("xt", [128, G_PER_CORE, KCH, 128])          # X^T chunked per graph
    at_d = din("at_d", [128, G_PER_CORE, 2, 128])        # normalized adj^T
    w1p = din("w1p", [128, KCH, 2 * H_FEATS])            # [W1_td | W1_bu]
    w2h = din("w2h", [128, 2, 2, H_FEATS])               # W2h per (branch, chunk)
    rvrow = din("rvrow", [1, G_PER_CORE * 2 * H_FEATS], F32)  # host rvec rows
    if has_bias:
        b1c = din("b1c", [128, 4], F32)                  # b1 per (branch,chunk)
        b2c = din("b2c", [128, 4], F32)                  # b2 per (branch,chunk)
    out = nc.dram_tensor("out", [G_PER_CORE, 4 * H_FEATS], F32,
                         kind="ExternalOutput").ap()

    with tile.TileContext(nc) as tc:
        with (
            tc.tile_pool(name="const", bufs=1) as const,
            tc.tile_pool(name="xin", bufs=3) as xin,
            tc.tile_pool(name="atp", bufs=3) as atp,
            tc.tile_pool(name="ybp", bufs=2) as ybp,
            tc.tile_pool(name="htp", bufs=2) as htp,
            tc.tile_pool(name="znp", bufs=2) as znp,
            tc.tile_pool(name="rvf", bufs=2) as rvf,
            tc.tile_pool(name="scr", bufs=2) as scr,
            tc.tile_pool(name="psA", bufs=2, space="PSUM") as psA,
            tc.tile_pool(name="psH", bufs=2, space="PSUM") as psH,
            tc.tile_pool(name="psZ", bufs=2, space="PSUM") as psZ,
            tc.tile_pool(name="psT", bufs=2, space="PSUM") as psT,
        ):
            # ---- constants -------------------------------------------------
            identity_f32 = const.tile([128, 128], F32)
            make_identity(nc, identity_f32[:])
            # warm the PE p-state while startup DMAs are in flight
            ps_warm = psT.tile([128, 4, 128], F32, tag="psT")
            for i in range(14):
                nc.tensor.transpose(ps_warm[:, i % 4, :], identity_f32[:],
                                    identity_f32[:])

            w1p_sb = const.tile([128, KCH, 2 * H_FEATS], MM_DT)
            nc.sync.dma_start(w1p_sb[:], w1p)
            w2h_sb = const.tile([128, 2, 2, H_FEATS], MM_DT)
            nc.sync.dma_start(w2h_sb[:], w2h)
            rvrow_sb = const.tile([1, G_PER_CORE * 2 * H_FEATS], F32)
            nc.sync.dma_start(rvrow_sb[:], rvrow)
            if has_bias:
                b1c_sb = const.tile([128, 4], F32)
                nc.sync.dma_start(b1c_sb[:], b1c)
                b2c_sb = const.tile([128, 4], F32)
                nc.sync.dma_start(b2c_sb[:], b2c)

            # per-graph readout stashes (feature-major), transposed and
            # flushed to DRAM in two halves (mid-loop and at the end)
            G = G_PER_CORE
            roots_sb = const.tile([128, 4, G_PER_CORE], F32)
            means_sb = const.tile([128, 4, G_PER_CORE], F32)
            flushT = {}

            def flush_readout(lo, hi):
                n = hi - lo
                ps_r = psH.tile([128, 4, 128], F32, tag="psH")
                ps_m = psT.tile([128, 4, 128], F32, tag="psT")
                for fo in range(4):
                    nc.tensor.transpose(ps_r[0:n, fo, :],
                                        roots_sb[:, fo, lo:hi], identity_f32[:])
                    nc.tensor.transpose(ps_m[0:n, fo, :],
                                        means_sb[:, fo, lo:hi], identity_f32[:])
                rootsT = const.tile([n, 4, 128], F32, name=f"rootsT{lo}")
                meansT = const.tile([n, 4, 128], F32, name=f"meansT{lo}")
                flushT[lo] = (rootsT, meansT)
                nc.scalar.copy(rootsT[:], ps_r[0:n])
                # sums are >= 0, so the plain scale gives the exact mean
                nc.vector.tensor_scalar(meansT[:], ps_m[0:n],
                                        1.0 / N_PER_G, None, OP.mult)
                nc.sync.dma_start(out[lo:hi, 0:H_FEATS], meansT[:, 0:2, :])
                nc.sync.dma_start(out[lo:hi, 2 * H_FEATS:3 * H_FEATS],
                                  meansT[:, 2:4, :])
                nc.sync.dma_start(out[lo:hi, H_FEATS:2 * H_FEATS],
                                  rootsT[:, 0:2, :])
                nc.sync.dma_start(out[lo:hi, 3 * H_FEATS:4 * H_FEATS],
                                  rootsT[:, 2:4, :])

            # ---- main loop over this core's graphs -------------------------
            for g in range(G_PER_CORE):
                xt_tile = xin.tile([128, KCH, 128], MM_DT, tag="xt")
                nc.sync.dma_start(xt_tile[:], xt[:, g])
                at_sb = atp.tile([128, 2, 128], MM_DT, tag="at")
                nc.sync.dma_start(at_sb[:], at_d[:, g])
                # rvec row -> all 128 partitions (GpSimd; SBUF->SBUF)
                rvfull = rvf.tile([128, 2 * H_FEATS], F32, tag="rvfull")
                nc.gpsimd.partition_broadcast(
                    rvfull[:], rvrow_sb[0:1, g * 2 * H_FEATS:(g + 1) * 2 * H_FEATS])

                # Y = X @ [W1_td | W1_bu]   (node-major [128, 512])
                ps_y = psA.tile([128, 2 * H_FEATS], F32, tag="psA")
                for k in range(KCH):
                    nc.tensor.matmul(ps_y[:], xt_tile[:, k, :], w1p_sb[:, k, :],
                                     start=(k == 0), stop=(k == KCH - 1))
                yb = ybp.tile([128, 2 * H_FEATS], MM_DT, tag="yb")
                nc.vector.tensor_copy(yb[:], ps_y[:])

                # h^T chunks = (Y chunk)^T @ At_b   (feature-major)
                ps_h = psH.tile([128, 4, 128], F32, tag="psH")
                for b in (0, 1):
                    for c in (0, 1):
                        fo = b * 2 + c
                        nc.tensor.matmul(ps_h[:, fo, :],
                                         yb[:, fo * 128:(fo + 1) * 128],
                                         at_sb[:, b, :])
                ht = htp.tile([128, 4, 128], MM_DT, tag="ht")
                if has_bias:
                    for fo in range(4):
                        nc.scalar.activation(ht[:, fo, :], ps_h[:, fo, :],
                                             AF.Relu, bias=b1c_sb[:, fo:fo + 1])
                else:
                    nc.scalar.activation(ht[:], ps_h[:], AF.Relu)
                # stash roots' h (feature-major: column 0 of each chunk)
                nc.gpsimd.tensor_copy(roots_sb[:, :, g:g + 1], ht[:, :, 0:1])

                # Z = h @ W2h   (node-major [128, 512]); rvec added in the cast
                ps_z = psZ.tile([128, 2 * H_FEATS], F32, tag="psZ")
                for b in (0, 1):
                    col = slice(b * H_FEATS, (b + 1) * H_FEATS)
                    for c in (0, 1):
                        nc.tensor.matmul(ps_z[:, col], ht[:, b * 2 + c, :],
                                         w2h_sb[:, b, c, :],
                                         start=(c == 0), stop=(c == 1))
                zn = znp.tile([128, 2 * H_FEATS], MM_DT, tag="zn")
                nc.vector.tensor_tensor(zn[:], ps_z[:], rvfull[:], OP.add)

                # H2^T chunks = (Z chunk)^T @ At_b  (feature-major); the relu
                # cast accumulates node sums per feature => mean readout free
                ps_t = psT.tile([128, 4, 128], F32, tag="psT")
                for b in (0, 1):
                    for c in (0, 1):
                        fo = b * 2 + c
                        nc.tensor.matmul(ps_t[:, fo, :],
                                         zn[:, fo * 128:(fo + 1) * 128],
                                         at_sb[:, b, :])
                h2s = scr.tile([128, 4, 128], MM_DT, tag="h2s")
                for fo in range(4):
                    if has_bias:
                        # bias+relu+sum needs two ALU stages before the
                        # reduce, which DVE can't express -> all on ACT
                        nc.scalar.activation(
                            h2s[:, fo, :], ps_t[:, fo, :], AF.Relu,
                            bias=b2c_sb[:, fo:fo + 1],
                            accum_out=means_sb[:, fo, g:g + 1])
                    elif fo < 2:
                        # DVE reduce form: out = (in0 op0 s1);
                        # accum_out = reduce(out, op1, init=s2)
                        nc.vector.tensor_scalar(
                            h2s[:, fo, :], ps_t[:, fo, :], 0.0, 0.0,
                            OP.max, OP.add,
                            accum_out=means_sb[:, fo, g:g + 1])
                    else:
                        nc.scalar.activation(
                            h2s[:, fo, :], ps_t[:, fo, :], AF.Relu,
                            accum_out=means_sb[:, fo, g:g + 1])
                if g == G // 2 - 1:
                    flush_readout(0, G // 2)
                elif g == G - 1:
                    flush_readout(G // 2, G)



    nc.compile()
    return nc


# ----------------------------------------------------------------------------
# Host entry point
# ----------------------------------------------------------------------------

def _prep(inputs, w1_td, b1_td, w2_td, b2_td, w1_bu, b1_bu, w2_bu, b2_bu,
          td_src, td_dst, bu_src, bu_dst, nodes_per_graph):
    n = int(nodes_per_graph)
    X = np.asarray(inputs, np.float32)
    N = X.shape[0]
    G = N // n
    assert (n, G, X.shape[1]) == (N_PER_G, N_GRAPHS, IN_FEATS), \
        f"unexpected shapes {X.shape} n={n}"

    at_td = build_adj(td_src, td_dst, n, G)   # [G, 128, 128] f32
    at_bu = build_adj(bu_src, bu_dst, n, G)

    w1p = np.concatenate([np.asarray(w1_td, np.float32),
                          np.asarray(w1_bu, np.float32)], axis=1)
    w1p_l = np.ascontiguousarray(
        w1p.reshape(KCH, 128, 2 * H_FEATS).transpose(1, 0, 2)).astype(np.float16)
    w2_td = np.asarray(w2_td, np.float32)
    w2_bu = np.asarray(w2_bu, np.float32)
    w2h = np.stack([w2_td[:H_FEATS].reshape(2, 128, H_FEATS),
                    w2_bu[:H_FEATS].reshape(2, 128, H_FEATS)], axis=0)
    w2h_l = np.ascontiguousarray(w2h.transpose(2, 0, 1, 3)).astype(np.float16)
    w2rp = np.concatenate([w2_td[H_FEATS:], w2_bu[H_FEATS:]], axis=1)  # [768,512]
    rv_all = (X[::n] @ w2rp).astype(np.float32)                        # [G, 512]

    biases = [np.asarray(b, np.float32) for b in (b1_td, b2_td, b1_bu, b2_bu)]
    has_bias = any(np.any(b != 0) for b in biases)

    in_maps = []
    for c in range(N_CORES):
        gs = slice(c * G_PER_CORE, (c + 1) * G_PER_CORE)
        ns = slice(c * NODES_PER_CORE, (c + 1) * NODES_PER_CORE)
        Xc = X[ns]
        xt_l = np.ascontiguousarray(
            Xc.reshape(G_PER_CORE, 128, KCH, 128).transpose(3, 0, 2, 1)
        ).astype(np.float16)
        at_c = np.stack([at_td[gs], at_bu[gs]], axis=0)  # [b, g, j, i]
        at_l = np.ascontiguousarray(at_c.transpose(2, 1, 0, 3)).astype(np.float16)
        m = {
            "xt": xt_l,
            "at_d": at_l,
            "w1p": w1p_l,
            "w2h": w2h_l,
            "rvrow": np.ascontiguousarray(rv_all[gs].reshape(1, -1)),
        }
        if has_bias:
            b1cat = np.concatenate([biases[0], biases[2]])       # [512]
            m["b1c"] = np.ascontiguousarray(b1cat.reshape(4, 128).T)
            b2cat = np.concatenate([biases[1], biases[3]])       # [512]
            m["b2c"] = np.ascontiguousarray(b2cat.reshape(4, 128).T)
        in_maps.append(m)
    return in_maps, has_bias


_PROGRAM_CACHE = {}


def _get_program(key):
    if key not in _PROGRAM_CACHE:
        _PROGRAM_CACHE[key] = build_program(key)
    return _PROGRAM_CACHE[key]


def kernel(trace=False, tmpdir=None, _return_raw=False, **inputs):
    in_maps, has_bias = _prep(**inputs)
    nc = _get_program(has_bias)
    res = run_bass_kernel_spmd(nc, in_maps, list(range(N_CORES)),
                               trace=trace, tmpdir=tmpdir)
    out = np.concatenate([res.results[i]["out"] for i in range(N_CORES)], axis=0)
    if _return_raw:
        return out, res
    return out
